# revision 52
# baseline (speedup 1.0000x reference)
"""Fused self-attention flow kernel for Trainium2 (8 NeuronCores), v3.

Problem (hardcoded): B=4, C=256, H=W=64, N=4096.
  x      = (inp [B,C,H,W] -> [B,N,C]) @ W_lin.T + b_lin
  scores = (x/16) @ x.T
  attn   = softmax(scores, -1)
  out    = (attn @ flow [B,N,2]) -> [B,2,H,W]

Sharding: core c handles batch c//2, q-half c%2; inputs rolled along N so
local q rows are 0..2047 (SPMD-identical program on all cores).

Device algorithm (per core):
  1. Linear in fp16 (PE); bias rides the ACT PSUM->fp8 copy as a
     per-partition bias AP (Identity activation; Copy rejects AP bias) --
     no rank-1 bias matmuls. Limbs: xh=fp8(x+b) (ACT), raw=pl-xh (DVE),
     xl=raw+b (Pool, SBUF->SBUF); Pool also computes xh2=2*xh.
     lin_ps PSUM banks are freed after phase A so phase B gets 3 score
     buffers (deeper PE<->exp pipeline).
  2. Scores in fp8 DoubleRow: s = xh_k . xh_q, one [128,2,512] PSUM tile
     per (k-pair j of 256, q-chunk c of 512). On the 8 diagonal windows
     (j<8, c=j//2) ONE extra correction matmul per half adds
     xl_k . (2*xh_q) -- exact on the k==q diagonal where the correlated
     quantization error of |x_q|^2 otherwise dominates; off-diagonal the
     2x one-sided term is just zero-mean noise (validated 6.4e-3).
  3. exp(s/16 - 1), fp16 out, split across ACT (true exp; all diag tiles
     + 25 others) and DVE (31 tiles, Schraudolph fast-exp: one
     tensor_scalar s*MUL16+ADD16 -> int16 whose integer result IS the
     fp16 bit pattern). The -1 shift cancels in num/den. Pool cannot
     read PSUM (HW), so it only gets SBUF->SBUF work (xh2).
  4. attn @ [f0,f1,1] TRANSPOSED: the fp16 exp tile is the STATIONARY
     operand (LDWEIGHTS+FWL ~31ns/matmul, overhead-bound) and the flow
     [128 k, 3] fp16 the 3-column moving operand, accumulating
     out[128 q, qb, 3] over all 32 k-blocks. 16 q-block accumulation
     groups run as CONTIGUOUS 32-matmul groups in one PSUM bank
     (interleaved groups in one bank corrupt PSUM accumulation state).
     This cuts PF from 27us (moving-side streaming at 1 col/cycle) to
     ~16us and avoids any fp8 quantization of attn or flow.
  5. Host divides num/den and unshards.

Perf notes (sustained, HAM-throttled regime; measured via R=65/129
NEFF-rep slope): baseline 131.5us -> 98us. PE-bound: scores 128 DR
matmuls ~300ns each; PF 512 small matmuls ~31ns each (instruction
overhead, independent of weight dtype/columns); linear fp16 ~433ns/MM.
fp8 PF weights measured no faster than fp16 -> keep fp16 precision.
"""

import os

import numpy as np

B, C, H, W = 4, 256, 64, 64
N = H * W            # 4096
QL = N // 2          # per-core q rows
PAIRS = 16           # k-pairs of 256
CHUNKS = 4           # q chunks of 512
NCORES = 8

SHIFT = -1.0
_LN2 = float(np.log(2.0))
MUL8 = 0.0625 * 8.0 / _LN2
ADD8 = 56.0 - 0.344 + SHIFT * 8.0 / _LN2
MUL16 = 0.0625 * 1024.0 / _LN2
ADD16 = 15360.0 - 44.0 + SHIFT * 1024.0 / _LN2

_CACHE = {}

# engine schedule for the 56 non-diag exp tiles (diag tiles always ACT/fp16)
# counts tuned for rate balance: ACT 1029ns/tile, Pool 889, DVE 1174
_N_ACT = int(os.environ.get("K_NACT", "25"))
_N_POOL = int(os.environ.get("K_NPOOL", "0"))
_CONV_POOL = int(os.environ.get("K_CONVPOOL", "22"))  # of 32 conv ops
_ABLATE = os.environ.get("K_ABLATE", "")  # noexp | nopf (timing-only ablations)


def _mk_schedule():
    """(j, c) -> 'act16' | 'act' | 'dve' for the 64 exp tiles.

    Greedy on cumulative engine time so the ACT-forced diag tiles (all at
    j<8) don't create local ACT bursts that stall the PE<->exp pipeline."""
    COST_A, COST_D = 1029.0, 1174.0
    sched = {}
    t_eng = {"act": 0.0, "dve": 0.0}
    for j in range(PAIRS):
        for c in range(CHUNKS):
            if j < 8 and c == j // 2:
                sched[(j, c)] = "act16"
                t_eng["act"] += COST_A
            elif t_eng["act"] + COST_A <= t_eng["dve"] + COST_D:
                sched[(j, c)] = "act"
                t_eng["act"] += COST_A
            else:
                sched[(j, c)] = "dve"
                t_eng["dve"] += COST_D
    return sched


def _build_body(nc, tc, ctx, mybir, dram):
    from contextlib import ExitStack

    f32 = mybir.dt.float32
    f16 = mybir.dt.float16
    f8 = mybir.dt.float8e4
    i16 = mybir.dt.int16
    i8 = mybir.dt.int8
    inp_d, wt_d, b32_d, flow16_d, out_d = dram

    sb = ctx.enter_context(tc.tile_pool(name="sb", bufs=1))
    pt16_pool = ctx.enter_context(tc.tile_pool(name="pt16", bufs=64))
    lin_ctx = ctx.enter_context(ExitStack())
    lin_ps = lin_ctx.enter_context(tc.tile_pool(name="lin_ps", bufs=2, space="PSUM"))
    # lin_ps (2 banks) is freed after phase A, making room for 3 score bufs
    # (sc_ps + out_ps are created after lin_ctx closes: LIFO pool order)

    sched = _mk_schedule()

    # --- constants / small inputs ---
    wt_sb = sb.tile([128, 2, C], f16)
    nc.sync.dma_start(out=wt_sb[:], in_=wt_d[:])
    b32_sb = sb.tile([128, 2], f32)
    nc.sync.dma_start(out=b32_sb[:], in_=b32_d[:])
    flow16_sb = sb.tile([128, 32, 3], f16)
    nc.sync.dma_start(out=flow16_sb[:], in_=flow16_d[:])

    shift_sb = sb.tile([128, 1], f32)
    nc.vector.memset(shift_sb[:], SHIFT)

    # warm up the exp table-load (~1.3us) under the input DMA
    warm = sb.tile([128, 8], f32)
    nc.vector.memset(warm[:], 0.0)
    nc.scalar.activation(out=warm[:], in_=warm[:],
                         func=mybir.ActivationFunctionType.Exp)

    # --- phase A: linear + 2-limb fp8 quantization ---
    inp_f16 = [sb.tile([128, N], f16, name=f"inp_f16_{ic}", tag=f"inpf{ic}")
               for ic in range(2)]
    xh8 = sb.tile([128, 2, N], f8, name="xh8", tag="xh8")
    xl8 = sb.tile([128, 2, QL], f8, name="xl8", tag="xl8")
    raw8 = sb.tile([128, 2, QL], f8, name="raw8", tag="raw8")
    # 2*xh over local q, computed on the otherwise-idle Pool engine
    # (SBUF->SBUF, Pool cannot touch PSUM); used by the single-sided diag
    # correction: s += xl_k . (2*xh_q), exact on the k==q diagonal.
    xh2 = sb.tile([128, 2, QL], f8, name="xh2", tag="xh2")

    for nt in range(8):
        s = slice(nt * 512, (nt + 1) * 512)
        for ic in range(2):
            nc.sync.dma_start(out=inp_f16[ic][:, s],
                              in_=inp_d[ic * 128:(ic + 1) * 128, s])
        for oc in range(2):
            pl = lin_ps.tile([128, 512], f32, name="pl", tag="pl")
            for ic in range(2):
                nc.tensor.matmul(
                    pl[:],
                    lhsT=wt_sb[:, ic, oc * 128:(oc + 1) * 128],
                    rhs=inp_f16[ic][:, s],
                    start=(ic == 0), stop=(ic == 1),
                )
            # bias rides the PSUM->fp8 copy (per-partition bias AP): no
            # rank-1 bias matmuls on the PE
            nc.scalar.activation(out=xh8[:, oc, s], in_=pl[:],
                                 func=mybir.ActivationFunctionType.Identity,
                                 bias=b32_sb[:, oc:oc + 1])
            if nt < 4:
                # raw = pl - xh = xl_true - b; Pool (idle, SBUF->SBUF)
                # re-adds b to recover the exact correction limb
                nc.vector.tensor_sub(raw8[:, oc, s], pl[:], xh8[:, oc, s])
                nc.gpsimd.tensor_scalar(
                    out=xl8[:, oc, s], in0=raw8[:, oc, s],
                    scalar1=b32_sb[:, oc:oc + 1], scalar2=0.0,
                    op0=mybir.AluOpType.add, op1=mybir.AluOpType.add,
                )
                nc.gpsimd.tensor_scalar(
                    out=xh2[:, oc, s], in0=xh8[:, oc, s],
                    scalar1=2.0, scalar2=0.0,
                    op0=mybir.AluOpType.mult, op1=mybir.AluOpType.add,
                )

    lin_ctx.close()                      # free lin_ps banks
    sc_ps = ctx.enter_context(tc.tile_pool(name="sc_ps", bufs=3, space="PSUM"))
    out_ps_pool = ctx.enter_context(tc.tile_pool(name="out_ps", bufs=1,
                                                 space="PSUM"))

    # --- phase B: scores + exp + PF ---
    # PF is transposed: attn tile is the stationary operand (LDWEIGHTS with
    # FWL), flow [128 k, 3] the moving operand; out_acc[128 q, qb, 3]
    # accumulates over all 32 k-blocks. 16 q-block accumulation groups.
    QB = QL // 128                       # 16 q-blocks of 128
    out_acc = out_ps_pool.tile([128, QB, 8], f32)
    pts_all = {}                         # (j, c) -> exp tile

    for j in range(PAIRS):
        for c in range(CHUNKS):
            qs = slice(c * 512, (c + 1) * 512)
            diag = sched[(j, c)] == "act16"
            eng = sched[(j, c)]
            if _ABLATE == "noexp" and (j, c) != (0, 0):
                pts_all[(j, c)] = pts_all[(0, 0)]
                continue
            pt = pt16_pool.tile([128, 2, 512], f16, name="pt16", tag="pt16")
            ps = sc_ps.tile([128, 2, 512], f32, name="ps", tag="ps")
            for i in range(2):
                ks = slice(j * 256 + i * 128, j * 256 + (i + 1) * 128)
                nc.tensor.matmul(
                    ps[:, i, :],
                    lhsT=xh8[:, :, ks],
                    rhs=xh8[:, :, qs],
                    start=True, stop=not diag,
                    perf_mode=mybir.MatmulPerfMode.DoubleRow,
                    skip_group_check=True,
                )
                if diag:
                    # correction only on the 256-q half containing the k==q
                    # diagonal; elsewhere it is zero-mean noise
                    qoff = 256 * (j % 2)
                    nc.tensor.matmul(
                        ps[:, i, qoff:qoff + 256],
                        lhsT=xl8[:, :, ks],
                        rhs=xh2[:, :, j * 256:(j + 1) * 256],
                        start=False, stop=True,
                        perf_mode=mybir.MatmulPerfMode.DoubleRow,
                        skip_group_check=True,
                    )
            if eng in ("act16", "act"):
                nc.scalar.activation(out=pt[:], in_=ps[:],
                                     func=mybir.ActivationFunctionType.Exp,
                                     scale=0.0625, bias=shift_sb[:])
            else:
                nc.vector.tensor_scalar(
                    out=pt[:].bitcast(i16), in0=ps[:],
                    scalar1=MUL16, scalar2=ADD16,
                    op0=mybir.AluOpType.mult, op1=mybir.AluOpType.add,
                )
            pts_all[(j, c)] = pt

    # PF tail: attn tiles as stationary operand; per q-block a contiguous
    # 32-matmul accumulation group over all k-blocks.
    for qb in range(QB if _ABLATE != "nopf" else 1):
        c, qq = divmod(qb, 4)
        for j in range(PAIRS):
            pt = pts_all[(j, c)]
            for i in range(2):
                nc.tensor.matmul(
                    out_acc[:, qb, 0:3],
                    lhsT=pt[:, i, qq * 128:(qq + 1) * 128],
                    rhs=flow16_sb[:, 2 * j + i, :],
                    start=(j == 0 and i == 0),
                    stop=(j == PAIRS - 1 and i == 1),
                    skip_group_check=True,
                )

    out_sb = sb.tile([128, QB, 3], f32)
    nc.vector.tensor_copy(out=out_sb[:], in_=out_acc[:, :, 0:3])
    nc.sync.dma_start(out=out_d[:], in_=out_sb[:])


def _build_nc(reps=1, **_unused):
    from contextlib import ExitStack

    import concourse.bacc as bacc
    import concourse.tile as tile
    from concourse import mybir

    f32 = mybir.dt.float32
    f16 = mybir.dt.float16
    f8 = mybir.dt.float8e4

    nc = bacc.Bacc("TRN2", target_bir_lowering=False, debug=False)

    dram = (
        nc.dram_tensor("inp", (C, N), f16, kind="ExternalInput"),
        nc.dram_tensor("wt", (128, 2, C), f16, kind="ExternalInput"),
        nc.dram_tensor("b32", (128, 2), f32, kind="ExternalInput"),
        nc.dram_tensor("flow16", (128, 32, 3), f16, kind="ExternalInput"),
        nc.dram_tensor("out", (128, 16, 3), f32, kind="ExternalOutput"),
    )

    with tile.TileContext(nc) as tc:
        for _ in range(reps):
            with ExitStack() as ctx:
                _build_body(nc, tc, ctx, mybir, dram)

    nc.compile()
    return nc


_FP8 = False   # kept for bench_hw compat
_FP8C = False


def _get_nc(reps=1):
    key = ("nc", reps)
    if key not in _CACHE:
        _CACHE[key] = _build_nc(reps)
    return _CACHE[key]


def _make_in_maps(inp, flow_init, W_lin, b_lin):
    import ml_dtypes
    F8 = ml_dtypes.float8_e4m3

    inp = np.ascontiguousarray(np.asarray(inp, dtype=np.float32)).reshape(B, C, N)
    flow = np.ascontiguousarray(np.asarray(flow_init, dtype=np.float32)).reshape(B, 2, N)
    W_lin = np.asarray(W_lin, dtype=np.float32)
    b_lin = np.asarray(b_lin, dtype=np.float32)

    # lhsT layout for x = W @ inp: [c_in(part 128), ic, c_out]
    wt = np.ascontiguousarray(
        W_lin.T.reshape(2, 128, C).transpose(1, 0, 2)).astype(np.float16)
    b32 = np.ascontiguousarray(b_lin.reshape(2, 128).T).astype(np.float32)

    in_maps = []
    for c in range(NCORES):
        b, half = divmod(c, 2)
        sh = -QL * half
        inp_c = np.roll(inp[b], sh, axis=1) if half else inp[b]
        inp_c = inp_c.astype(np.float16)
        f = np.roll(flow[b], sh, axis=1) if half else flow[b]
        flow3 = np.empty((N, 3), np.float32)
        flow3[:, 0:2] = f.T
        flow3[:, 2] = 1.0
        flow16 = np.ascontiguousarray(
            flow3.reshape(32, 128, 3).transpose(1, 0, 2)).astype(np.float16)
        in_maps.append({
            "inp": np.ascontiguousarray(inp_c),
            "wt": wt,
            "b32": b32,
            "flow16": flow16,
        })
    return in_maps


def _postprocess(results):
    out = np.empty((B, 2, N), np.float32)
    for c in range(NCORES):
        b, half = divmod(c, 2)
        acc = results[c]["out"].reshape(128, 16, 3)
        a = acc.transpose(1, 0, 2).reshape(QL, 3)                # [2048, 3]
        out[b, :, half * QL:(half + 1) * QL] = (a[:, 0:2] / a[:, 2:3]).T
    return out.reshape(B, 2, H, W)


def _run(inputs, trace=False):
    from concourse.bass_utils import run_bass_kernel_spmd

    nc = _get_nc()
    in_maps = _make_in_maps(inputs["inp"], inputs["flow_init"],
                            inputs["W_lin"], inputs["b_lin"])
    r = run_bass_kernel_spmd(nc, in_maps, core_ids=list(range(NCORES)),
                             trace=False)
    _CACHE["last_exec_ns"] = r.exec_time_ns
    return _postprocess(r.results)


def kernel(**inputs) -> np.ndarray:
    return _run(inputs, trace=False)



# revision 56
# speedup vs baseline: 1.0115x; 1.0115x over previous
"""Fused self-attention flow kernel for Trainium2 (8 NeuronCores), v3.

Problem (hardcoded): B=4, C=256, H=W=64, N=4096.
  x      = (inp [B,C,H,W] -> [B,N,C]) @ W_lin.T + b_lin
  scores = (x/16) @ x.T
  attn   = softmax(scores, -1)
  out    = (attn @ flow [B,N,2]) -> [B,2,H,W]

Sharding: core c handles batch c//2, q-half c%2; inputs rolled along N so
local q rows are 0..2047 (SPMD-identical program on all cores).

Device algorithm (per core):
  1. Linear in fp16 (PE); bias rides the ACT PSUM->fp8 copy as a
     per-partition bias AP (Identity activation; Copy rejects AP bias) --
     no rank-1 bias matmuls. Limbs: xh=fp8(x+b) (ACT), raw=pl-xh (DVE),
     xl=raw+b (Pool, SBUF->SBUF); Pool also computes xh2=2*xh.
     lin_ps PSUM banks are freed after phase A so phase B gets 3 score
     buffers (deeper PE<->exp pipeline).
  2. Scores in fp8 DoubleRow: s = xh_k . xh_q, one [128,2,512] PSUM tile
     per (k-pair j of 256, q-chunk c of 512). On the 8 diagonal windows
     (j<8, c=j//2) ONE extra correction matmul per half adds
     xl_k . (2*xh_q) -- exact on the k==q diagonal where the correlated
     quantization error of |x_q|^2 otherwise dominates; off-diagonal the
     2x one-sided term is just zero-mean noise (validated 6.4e-3).
  3. exp(s/16 - 1), fp16 out, split across ACT (true exp; all diag tiles
     + 25 others) and DVE (31 tiles, Schraudolph fast-exp: one
     tensor_scalar s*MUL16+ADD16 -> int16 whose integer result IS the
     fp16 bit pattern). The -1 shift cancels in num/den. Pool cannot
     read PSUM (HW), so it only gets SBUF->SBUF work (xh2).
  4. attn @ [f0,f1,1] TRANSPOSED: the fp16 exp tile is the STATIONARY
     operand (LDWEIGHTS+FWL ~31ns/matmul, overhead-bound) and the flow
     [128 k, 3] fp16 the 3-column moving operand, accumulating
     out[128 q, qb, 3] over all 32 k-blocks. 16 q-block accumulation
     groups run as CONTIGUOUS 32-matmul groups in one PSUM bank
     (interleaved groups in one bank corrupt PSUM accumulation state).
     This cuts PF from 27us (moving-side streaming at 1 col/cycle) to
     ~16us and avoids any fp8 quantization of attn or flow.
  5. Host divides num/den and unshards.

Perf notes (sustained, HAM-throttled regime; measured via R=65/129
NEFF-rep slope): baseline 131.5us -> 98us. PE-bound: scores 128 DR
matmuls ~300ns each; PF 512 small matmuls ~31ns each (instruction
overhead, independent of weight dtype/columns); linear fp16 ~433ns/MM.
fp8 PF weights measured no faster than fp16 -> keep fp16 precision.
"""

import os

import numpy as np

B, C, H, W = 4, 256, 64, 64
N = H * W            # 4096
QL = N // 2          # per-core q rows
PAIRS = 16           # k-pairs of 256
CHUNKS = 4           # q chunks of 512
NCORES = 8

SHIFT = -1.0
_LN2 = float(np.log(2.0))
MUL8 = 0.0625 * 8.0 / _LN2
ADD8 = 56.0 - 0.344 + SHIFT * 8.0 / _LN2
MUL16 = 0.0625 * 1024.0 / _LN2
ADD16 = 15360.0 - 44.0 + SHIFT * 1024.0 / _LN2

_CACHE = {}

# engine schedule for the 56 non-diag exp tiles (diag tiles always ACT/fp16)
# counts tuned for rate balance: ACT 1029ns/tile, Pool 889, DVE 1174
_N_ACT = int(os.environ.get("K_NACT", "25"))
_N_POOL = int(os.environ.get("K_NPOOL", "0"))
_CONV_POOL = int(os.environ.get("K_CONVPOOL", "22"))  # of 32 conv ops
_ABLATE = os.environ.get("K_ABLATE", "")  # noexp | nopf (timing-only ablations)


def _mk_schedule():
    """(j, c) -> 'act16' | 'act' | 'dve' for the 64 exp tiles.

    Greedy on cumulative engine time so the ACT-forced diag tiles (all at
    j<8) don't create local ACT bursts that stall the PE<->exp pipeline."""
    COST_A, COST_D = 1029.0, 1174.0
    sched = {}
    t_eng = {"act": 0.0, "dve": 0.0}
    for j in range(PAIRS):
        for c in range(CHUNKS):
            if j < 8 and c == j // 2:
                sched[(j, c)] = "act16"
                t_eng["act"] += COST_A
            elif t_eng["act"] + COST_A <= t_eng["dve"] + COST_D:
                sched[(j, c)] = "act"
                t_eng["act"] += COST_A
            else:
                sched[(j, c)] = "dve"
                t_eng["dve"] += COST_D
    return sched


def _build_body(nc, tc, ctx, mybir, dram):
    from contextlib import ExitStack

    f32 = mybir.dt.float32
    f16 = mybir.dt.float16
    f8 = mybir.dt.float8e4
    i16 = mybir.dt.int16
    i8 = mybir.dt.int8
    inp_d, wt_d, b32_d, flow16_d, out_d = dram

    sb = ctx.enter_context(tc.tile_pool(name="sb", bufs=1))
    pt16_pool = ctx.enter_context(tc.tile_pool(name="pt16", bufs=64))
    lin_ctx = ctx.enter_context(ExitStack())
    lin_ps = lin_ctx.enter_context(tc.tile_pool(name="lin_ps", bufs=2, space="PSUM"))
    # lin_ps (2 banks) is freed after phase A, making room for 3 score bufs
    # (sc_ps + out_ps are created after lin_ctx closes: LIFO pool order)

    sched = _mk_schedule()

    # --- constants / small inputs ---
    wt_sb = sb.tile([128, 2, C], f16)
    nc.sync.dma_start(out=wt_sb[:], in_=wt_d[:])
    b32_sb = sb.tile([128, 2], f32)
    nc.sync.dma_start(out=b32_sb[:], in_=b32_d[:])
    flow16_sb = sb.tile([128, 32, 3], f16)
    nc.sync.dma_start(out=flow16_sb[:], in_=flow16_d[:])

    shift_sb = sb.tile([128, 1], f32)
    nc.vector.memset(shift_sb[:], SHIFT)

    # warm up the exp table-load (~1.3us) under the input DMA
    warm = sb.tile([128, 8], f32)
    nc.vector.memset(warm[:], 0.0)
    nc.scalar.activation(out=warm[:], in_=warm[:],
                         func=mybir.ActivationFunctionType.Exp)

    # --- phase A: linear + 2-limb fp8 quantization ---
    inp_f16 = [sb.tile([128, N], f16, name=f"inp_f16_{ic}", tag=f"inpf{ic}")
               for ic in range(2)]
    xh8 = sb.tile([128, 2, N], f8, name="xh8", tag="xh8")
    xl8 = sb.tile([128, 2, QL], f8, name="xl8", tag="xl8")
    raw8 = sb.tile([128, 2, QL], f8, name="raw8", tag="raw8")
    # 2*xh over local q, computed on the otherwise-idle Pool engine
    # (SBUF->SBUF, Pool cannot touch PSUM); used by the single-sided diag
    # correction: s += xl_k . (2*xh_q), exact on the k==q diagonal.
    xh2 = sb.tile([128, 2, QL], f8, name="xh2", tag="xh2")

    for nt in range(8):
        s = slice(nt * 512, (nt + 1) * 512)
        for ic in range(2):
            nc.sync.dma_start(out=inp_f16[ic][:, s],
                              in_=inp_d[ic * 128:(ic + 1) * 128, s])
        for oc in range(2):
            pl = lin_ps.tile([128, 512], f32, name="pl", tag="pl")
            for ic in range(2):
                nc.tensor.matmul(
                    pl[:],
                    lhsT=wt_sb[:, ic, oc * 128:(oc + 1) * 128],
                    rhs=inp_f16[ic][:, s],
                    start=(ic == 0), stop=(ic == 1),
                )
            # bias rides the PSUM->fp8 copy (per-partition bias AP): no
            # rank-1 bias matmuls on the PE
            nc.scalar.activation(out=xh8[:, oc, s], in_=pl[:],
                                 func=mybir.ActivationFunctionType.Identity,
                                 bias=b32_sb[:, oc:oc + 1])
            if nt < 4:
                # raw = pl - xh = xl_true - b; Pool (idle, SBUF->SBUF)
                # re-adds b to recover the exact correction limb
                nc.vector.tensor_sub(raw8[:, oc, s], pl[:], xh8[:, oc, s])
                nc.gpsimd.tensor_scalar(
                    out=xl8[:, oc, s], in0=raw8[:, oc, s],
                    scalar1=b32_sb[:, oc:oc + 1], scalar2=0.0,
                    op0=mybir.AluOpType.add, op1=mybir.AluOpType.add,
                )
                nc.gpsimd.tensor_scalar(
                    out=xh2[:, oc, s], in0=xh8[:, oc, s],
                    scalar1=2.0, scalar2=0.0,
                    op0=mybir.AluOpType.mult, op1=mybir.AluOpType.add,
                )

    lin_ctx.close()                      # free lin_ps banks
    sc_ps = ctx.enter_context(tc.tile_pool(name="sc_ps", bufs=3, space="PSUM"))
    out_ps_pool = ctx.enter_context(tc.tile_pool(name="out_ps", bufs=1,
                                                 space="PSUM"))

    # --- phase B: scores + exp + PF ---
    # PF is transposed: attn tile is the stationary operand (LDWEIGHTS with
    # FWL), flow [128 k, 3] the moving operand; out_acc[128 q, qb, 3]
    # accumulates over all 32 k-blocks. 16 q-block accumulation groups.
    QB = QL // 128                       # 16 q-blocks of 128
    out_acc = out_ps_pool.tile([128, QB, 8], f32)
    pts_all = {}                         # (j, c) -> exp tile

    for j in range(PAIRS):
        for c in range(CHUNKS):
            qs = slice(c * 512, (c + 1) * 512)
            diag = sched[(j, c)] == "act16"
            eng = sched[(j, c)]
            if _ABLATE == "noexp" and (j, c) != (0, 0):
                pts_all[(j, c)] = pts_all[(0, 0)]
                continue
            pt = pt16_pool.tile([128, 2, 512], f16, name="pt16", tag="pt16")
            ps = sc_ps.tile([128, 2, 512], f32, name="ps", tag="ps")
            for i in range(2):
                ks = slice(j * 256 + i * 128, j * 256 + (i + 1) * 128)
                nc.tensor.matmul(
                    ps[:, i, :],
                    lhsT=xh8[:, :, ks],
                    rhs=xh8[:, :, qs],
                    start=True, stop=not diag,
                    perf_mode=mybir.MatmulPerfMode.DoubleRow,
                    skip_group_check=True,
                )
                if diag:
                    # correction only on the 256-q half containing the k==q
                    # diagonal; elsewhere it is zero-mean noise
                    qoff = 256 * (j % 2)
                    nc.tensor.matmul(
                        ps[:, i, qoff:qoff + 256],
                        lhsT=xl8[:, :, ks],
                        rhs=xh2[:, :, j * 256:(j + 1) * 256],
                        start=False, stop=True,
                        perf_mode=mybir.MatmulPerfMode.DoubleRow,
                        skip_group_check=True,
                    )
            if eng in ("act16", "act"):
                nc.scalar.activation(out=pt[:], in_=ps[:],
                                     func=mybir.ActivationFunctionType.Exp,
                                     scale=0.0625, bias=shift_sb[:])
            else:
                nc.vector.tensor_scalar(
                    out=pt[:].bitcast(i16), in0=ps[:],
                    scalar1=MUL16, scalar2=ADD16,
                    op0=mybir.AluOpType.mult, op1=mybir.AluOpType.add,
                )
            pts_all[(j, c)] = pt

    # PF tail: attn tiles as stationary operand; per q-block a contiguous
    # 32-matmul accumulation group over all k-blocks.
    for qb in range(QB if _ABLATE != "nopf" else 1):
        c, qq = divmod(qb, 4)
        for j in range(PAIRS):
            pt = pts_all[(j, c)]
            for i in range(2):
                nc.tensor.matmul(
                    out_acc[:, qb, 0:3],
                    lhsT=pt[:, i, qq * 128:(qq + 1) * 128],
                    rhs=flow16_sb[:, 2 * j + i, :],
                    start=(j == 0 and i == 0),
                    stop=(j == PAIRS - 1 and i == 1),
                    skip_group_check=True,
                )

    out_sb = sb.tile([128, QB, 3], f32)
    nc.vector.tensor_copy(out=out_sb[:], in_=out_acc[:, :, 0:3])
    nc.sync.dma_start(out=out_d[:], in_=out_sb[:])


def _build_nc(reps=1, **_unused):
    from contextlib import ExitStack

    import concourse.bacc as bacc
    import concourse.tile as tile
    from concourse import mybir

    f32 = mybir.dt.float32
    f16 = mybir.dt.float16
    f8 = mybir.dt.float8e4

    nc = bacc.Bacc("TRN2", target_bir_lowering=False, debug=False)

    dram = (
        nc.dram_tensor("inp", (C, N), f16, kind="ExternalInput"),
        nc.dram_tensor("wt", (128, 2, C), f16, kind="ExternalInput"),
        nc.dram_tensor("b32", (128, 2), f32, kind="ExternalInput"),
        nc.dram_tensor("flow16", (128, 32, 3), f16, kind="ExternalInput"),
        nc.dram_tensor("out", (128, 16, 3), f32, kind="ExternalOutput"),
    )

    with tile.TileContext(nc) as tc:
        for _ in range(reps):
            with ExitStack() as ctx:
                _build_body(nc, tc, ctx, mybir, dram)

    nc.compile()
    return nc


_FP8 = False   # kept for bench_hw compat
_FP8C = False


def _get_nc(reps=1):
    key = ("nc", reps)
    if key not in _CACHE:
        _CACHE[key] = _build_nc(reps)
    return _CACHE[key]


def _make_in_maps(inp, flow_init, W_lin, b_lin):
    import ml_dtypes
    F8 = ml_dtypes.float8_e4m3

    inp = np.ascontiguousarray(np.asarray(inp, dtype=np.float32)).reshape(B, C, N)
    flow = np.ascontiguousarray(np.asarray(flow_init, dtype=np.float32)).reshape(B, 2, N)
    W_lin = np.asarray(W_lin, dtype=np.float32)
    b_lin = np.asarray(b_lin, dtype=np.float32)

    # lhsT layout for x = W @ inp: [c_in(part 128), ic, c_out]
    wt = np.ascontiguousarray(
        W_lin.T.reshape(2, 128, C).transpose(1, 0, 2)).astype(np.float16)
    b32 = np.ascontiguousarray(b_lin.reshape(2, 128).T).astype(np.float32)

    in_maps = []
    for c in range(NCORES):
        b, half = divmod(c, 2)
        sh = -QL * half
        inp_c = np.roll(inp[b], sh, axis=1) if half else inp[b]
        inp_c = inp_c.astype(np.float16)
        f = np.roll(flow[b], sh, axis=1) if half else flow[b]
        flow3 = np.empty((N, 3), np.float32)
        flow3[:, 0:2] = f.T
        flow3[:, 2] = 1.0
        flow16 = np.ascontiguousarray(
            flow3.reshape(32, 128, 3).transpose(1, 0, 2)).astype(np.float16)
        in_maps.append({
            "inp": np.ascontiguousarray(inp_c),
            "wt": wt,
            "b32": b32,
            "flow16": flow16,
        })
    return in_maps


def _postprocess(results):
    out = np.empty((B, 2, N), np.float32)
    for c in range(NCORES):
        b, half = divmod(c, 2)
        acc = results[c]["out"].reshape(128, 16, 3)
        a = acc.transpose(1, 0, 2).reshape(QL, 3)                # [2048, 3]
        out[b, :, half * QL:(half + 1) * QL] = (a[:, 0:2] / a[:, 2:3]).T
    return out.reshape(B, 2, H, W)


def _run(inputs, trace=False):
    from concourse.bass_utils import run_bass_kernel_spmd

    nc = _get_nc()
    in_maps = _make_in_maps(inputs["inp"], inputs["flow_init"],
                            inputs["W_lin"], inputs["b_lin"])
    r = run_bass_kernel_spmd(nc, in_maps, core_ids=list(range(NCORES)),
                             trace=False)
    _CACHE["last_exec_ns"] = r.exec_time_ns
    return _postprocess(r.results)


def kernel(**inputs) -> np.ndarray:
    return _run(inputs, trace=False)

